# revision 13
# baseline (speedup 1.0000x reference)
"""Bahdanau attention Trainium2 Bass kernel.

Problem: B=16, S=8192, Q_DIM=K_DIM=ATTN_DIM=512 (fp32).
    q = query @ Wq; k = keys @ Wk
    e = tanh(q + k) @ v;  e = where(mask==0, -inf, e)
    a = softmax(e);  c = a @ keys;  returns (c, a)

Sharding: data-parallel over batch. 8 cores x 2 batch rows each, weights
replicated. Each core computes unnormalized softmax numerators
u = exp(e_masked) (exact 0 at masked positions), partial sums z, and the
unnormalized context c_un = sum_s u_s keys_s. Final normalization (divide
by Z) happens on the host -- the cross-shard combine of the hinted design.

Matmuls run in fp32r (~tf32 precision, ~1.5 cycles/row warm). The BIR
verifier requires fp32r operands to come from rounding ops, so DMA'd keys
pass through a DVE round-copy; ACT writes tanh as fp32r directly.

HAM note: PE-transpose mode does not count as "busy" for the PE clock
gate, so a contiguous block of 16 transposes (~7us) re-throttles the PE
to 1.2 GHz every group. The group loop is software-pipelined: the
transposes of group g+1 are interleaved between the projection matmul
bursts of group g so every HAM window sees real matmul activity.

Device algorithm per core, per batch row, per group of 512 seq rows:
  - DMA 4 keys tiles [128, 512]; DVE round-copy -> fp32r
  - PE transpose -> keysT [4][128k, 512r]  (interleaved, see above)
  - proj psum[128a, 512r] = sum_kc Wk_chunk.T @ keysT_chunk
  - ACT tanh(psum + bias=qT[:, ac, b]) -> SBUF fp32r   (bias per-partition)
  - score psum_e[1,512] = sum_ac v_ac.T @ tanh_ac + ones.T @ maskbias(bf16)
  - ACT exp(psum_e) -> u tile, accum_out -> z partial; DMA u out
  - uT via 4 tiny fp32 transposes; DVE round-copy
  - c: psum_c[1,512] += uT_t.T @ keys_t   (accumulates over whole batch)
"""

import os

import numpy as np

B, S, D = 16, 8192, 512
NCORES = 8
BPC = B // NCORES        # batches per core
GROUP = 512              # seq rows per group
CAP = 4608               # capacity of gathered (unmasked) rows per batch
NG = CAP // GROUP        # 9 groups per batch
if os.environ.get("NG_OVERRIDE"):
    NG = int(os.environ["NG_OVERRIDE"])  # debug: process fewer groups
T = GROUP // 128         # 4 keys tiles per group
KC = D // 128            # 4 contraction chunks
AC = D // 128            # 4 attn-dim chunks
NEG = -1.0e30

_CACHE: dict = {}


def _build():
    if "nc" in _CACHE:
        return _CACHE["nc"]

    import concourse.bass as bass
    import concourse.mybir as mybir
    import concourse.tile as tile
    from concourse import bacc
    from concourse.masks import make_identity

    f32 = mybir.dt.float32
    f32r = mybir.dt.float32r
    bf16 = mybir.dt.bfloat16
    AF = mybir.ActivationFunctionType

    nc = bacc.Bacc("TRN2", target_bir_lowering=False, debug=False)

    i32 = mybir.dt.int32
    keys_d = nc.dram_tensor("keys", [BPC, S, D], f32r, kind="ExternalInput").ap()
    keys_flat = keys_d.rearrange("b s d -> (b s) d")
    qT_d = nc.dram_tensor("queryT", [D, BPC], f32, kind="ExternalInput").ap()
    wq_d = nc.dram_tensor("Wq", [D, D], f32, kind="ExternalInput").ap()
    wk_d = nc.dram_tensor("Wk", [D, D], f32r, kind="ExternalInput").ap()
    v_d = nc.dram_tensor("v", [D], f32r, kind="ExternalInput").ap()
    # idxT[b, p, j] = global row index (into keys_flat) of gathered row
    # j*128 + p of batch b; padding points at row b*S with maskbias -1e30.
    idx_d = nc.dram_tensor("idxT", [BPC, 128, CAP // 128], i32, kind="ExternalInput").ap()
    mb_d = nc.dram_tensor("maskbias", [BPC, CAP], bf16, kind="ExternalInput").ap()
    u_d = nc.dram_tensor("u", [BPC, CAP], f32, kind="ExternalOutput").ap()
    z_d = nc.dram_tensor("z", [BPC, NG], f32, kind="ExternalOutput").ap()
    c_d = nc.dram_tensor("cun", [BPC, D], f32, kind="ExternalOutput").ap()

    with tile.TileContext(nc) as tc:
        with (
            tc.tile_pool(name="singles", bufs=1) as singles,
            tc.tile_pool(name="keysp", bufs=10) as keysp,
            tc.tile_pool(name="ktp", bufs=10) as ktp,
            tc.tile_pool(name="thp", bufs=8) as thp,
            tc.tile_pool(name="mbp", bufs=4) as mbp,
            tc.tile_pool(name="utp", bufs=3) as utp,
            tc.tile_pool(name="up", bufs=4) as up,
            tc.tile_pool(name="zp", bufs=2) as zp,
            tc.tile_pool(name="coutp", bufs=2) as coutp,
            tc.tile_pool(name="idxp", bufs=2) as idxp,
            tc.tile_pool(name="ps_t", bufs=2, space="PSUM") as ps_t,
            tc.tile_pool(name="ps_p", bufs=2, space="PSUM") as ps_p,
            tc.tile_pool(name="ps_e", bufs=2, space="PSUM") as ps_e,
            tc.tile_pool(name="ps_u", bufs=1, space="PSUM") as ps_u,
            tc.tile_pool(name="ps_c", bufs=1, space="PSUM") as ps_c,
        ):
            # ---- preload constants ----
            ident = singles.tile([128, 128], f32)
            make_identity(nc, ident)
            ident_r = singles.tile([128, 128], f32r)
            nc.vector.tensor_copy(ident_r, ident)
            ones_bf = singles.tile([1, 1], bf16)
            nc.vector.memset(ones_bf, 1.0)

            wk_r = singles.tile([128, KC, D], f32r)
            nc.sync.dma_start(out=wk_r, in_=wk_d.rearrange("(kc p) a -> p kc a", p=128))
            wq_sb = singles.tile([128, KC, D], f32)
            nc.sync.dma_start(out=wq_sb, in_=wq_d.rearrange("(kc p) a -> p kc a", p=128))
            qTd_sb = singles.tile([128, KC, BPC], f32)
            nc.sync.dma_start(out=qTd_sb, in_=qT_d.rearrange("(kc p) b -> p kc b", p=128))
            v_r = singles.tile([128, AC], f32r)
            nc.sync.dma_start(out=v_r, in_=v_d.rearrange("(ac p) -> p ac", p=128))

            # ---- q projection (fp32, tiny): qT[a, b] = sum_k Wq[k,a] query[b,k] ----
            ps_q = ps_p.tile([128, AC * BPC], f32, tag="kp")
            for ac in range(AC):
                for kc in range(KC):
                    nc.tensor.matmul(
                        ps_q[:, ac * BPC:(ac + 1) * BPC],
                        lhsT=wq_sb[:, kc, ac * 128:(ac + 1) * 128],
                        rhs=qTd_sb[:, kc, :],
                        start=(kc == 0),
                        stop=(kc == KC - 1),
                    )
            qT_sb = singles.tile([128, AC * BPC], f32)
            nc.vector.tensor_copy(qT_sb, ps_q)

            # ---- pipelined main loop ----
            def emit_load(b, g, idx_sb):
                """Indirect-gather keys tiles + DVE round-cast for group g."""
                r0 = g * GROUP
                kr = []
                for t in range(T):
                    kw = keysp.tile([128, D], f32r, tag="keys", name=f"kw{b}_{g}_{t}")
                    nc.gpsimd.indirect_dma_start(
                        out=kw,
                        out_offset=None,
                        in_=keys_flat,
                        in_offset=bass.IndirectOffsetOnAxis(
                            ap=idx_sb[:, g * T + t:g * T + t + 1], axis=0
                        ),
                    )
                    kr.append(kw)
                mb = mbp.tile([1, GROUP], bf16, tag="mb", name=f"mb{b}_{g}")
                nc.sync.dma_start(out=mb, in_=mb_d[b, None, r0:r0 + GROUP])
                return kr, mb

            def emit_transpose_block(kr, kc, b, g):
                """Transpose [128, 128] blocks of chunk kc -> kT SBUF tile."""
                pst = ps_t.tile([128, GROUP], f32r, tag="kT", name=f"pst{b}_{g}_{kc}")
                for t in range(T):
                    nc.tensor.transpose(
                        pst[:, t * 128:(t + 1) * 128],
                        kr[t][:, kc * 128:(kc + 1) * 128],
                        ident_r,
                    )
                kT = ktp.tile([128, GROUP], f32r, tag="kT_sb", name=f"kT{b}_{g}_{kc}")
                nc.vector.tensor_copy(kT, pst)
                return kT

            for b in range(BPC):
                z_sb = zp.tile([1, NG], f32, tag="z")
                psc = ps_c.tile([1, D], f32, tag="c")
                idx_sb = idxp.tile([128, CAP // 128], i32, tag="idx", name=f"idx{b}")
                nc.sync.dma_start(out=idx_sb, in_=idx_d[b])

                state = {}
                # prologue: load + transpose group 0
                state["load"] = emit_load(b, 0, idx_sb)
                state["kT"] = [
                    emit_transpose_block(state["load"][0], kc, b, 0) for kc in range(KC)
                ]

                for g in range(NG):
                    kr, mb = state["load"]
                    kTt = state["kT"]
                    if g + 1 < NG:
                        next_load = emit_load(b, g + 1, idx_sb)
                    else:
                        next_load = None
                    next_kT = []

                    # projection + tanh, with next group's transposes interleaved
                    th_tiles = []
                    for ac in range(AC):
                        if next_load is not None:
                            next_kT.append(
                                emit_transpose_block(next_load[0], ac, b, g + 1)
                            )
                        psp = ps_p.tile([128, GROUP], f32, tag="kp", name=f"kp{b}_{g}_{ac}")
                        for kc in range(KC):
                            nc.tensor.matmul(
                                psp,
                                lhsT=wk_r[:, kc, ac * 128:(ac + 1) * 128],
                                rhs=kTt[kc],
                                start=(kc == 0),
                                stop=(kc == KC - 1),
                            )
                        th = thp.tile([128, GROUP], f32r, tag="th", name=f"th{b}_{g}_{ac}")
                        nc.scalar.activation(
                            out=th, in_=psp, func=AF.Tanh,
                            bias=qT_sb[:, ac * BPC + b:ac * BPC + b + 1], scale=1.0,
                        )
                        th_tiles.append(th)

                    # scores
                    pse = ps_e.tile([1, GROUP], f32, tag="e", name=f"e{b}_{g}")
                    for ac in range(AC):
                        nc.tensor.matmul(
                            pse,
                            lhsT=v_r[:, ac:ac + 1],
                            rhs=th_tiles[ac],
                            start=(ac == 0),
                            stop=False,
                        )
                    nc.tensor.matmul(
                        pse, lhsT=ones_bf, rhs=mb,
                        start=False, stop=True, skip_group_check=True,
                    )

                    # u = exp(e) -> DMA out; z partial via accumulator
                    u_g = up.tile([1, GROUP], f32, tag="u", name=f"u{b}_{g}")
                    nc.scalar.activation(
                        out=u_g, in_=pse, func=AF.Exp,
                        accum_out=z_sb[:, g:g + 1],
                    )
                    nc.sync.dma_start(
                        out=u_d[b, None, g * GROUP:(g + 1) * GROUP], in_=u_g
                    )

                    # transpose u (tiny fp32 transposes) and accumulate context
                    psu = ps_u.tile([128, T], f32, tag="uT", name=f"uT{b}_{g}")
                    for t in range(T):
                        nc.tensor.transpose(
                            psu[:, t:t + 1],
                            u_g[:, t * 128:(t + 1) * 128],
                            ident[0:1, 0:1],
                        )
                    uT = utp.tile([128, T], f32r, tag="uT_sb", name=f"uTs{b}_{g}")
                    nc.vector.tensor_copy(uT, psu)
                    for t in range(T):
                        nc.tensor.matmul(
                            psc,
                            lhsT=uT[:, t:t + 1],
                            rhs=kr[t],
                            start=(g == 0 and t == 0),
                            stop=(g == NG - 1 and t == T - 1),
                        )

                    state["load"] = next_load
                    state["kT"] = next_kT

                # ---- batch epilogue ----
                c_sb = coutp.tile([1, D], f32, tag="cout")
                nc.vector.tensor_copy(c_sb, psc)
                nc.sync.dma_start(out=c_d[b, None, :], in_=c_sb)
                nc.sync.dma_start(out=z_d[b, None, :], in_=z_sb)

    nc.compile()
    _CACHE["nc"] = nc
    return nc


def kernel(query, keys, mask, Wq, Wk, v):
    import ml_dtypes
    from concourse import bass_utils

    query = np.asarray(query, dtype=np.float32)
    keys = np.asarray(keys, dtype=np.float32)
    mask = np.asarray(mask)
    Wq = np.ascontiguousarray(np.asarray(Wq, dtype=np.float32))
    Wk = np.ascontiguousarray(np.asarray(Wk, dtype=np.float32))
    v = np.ascontiguousarray(np.asarray(v, dtype=np.float32))

    # Gather plan: per batch row, the indices of unmasked positions, padded
    # to CAP with position 0 (given maskbias -1e30 so its u contribution is
    # exactly 0). Masked positions contribute exp(-inf)=0 in the reference
    # softmax, so skipping them is exact.
    idx_list, counts = [], []
    for j in range(B):
        nz = np.nonzero(mask[j])[0].astype(np.int32)
        n = len(nz)
        assert n <= CAP, f"unmasked count {n} exceeds kernel capacity {CAP}"
        counts.append(n)
        idx_list.append(np.pad(nz, (0, CAP - n)))
    idx = np.stack(idx_list)                                   # [B, CAP]
    mb_sp = np.zeros((B, CAP), dtype=ml_dtypes.bfloat16)
    for j in range(B):
        mb_sp[j, counts[j]:] = np.float32(NEG)

    nc = _build()

    in_maps = []
    for i in range(NCORES):
        sl = slice(i * BPC, (i + 1) * BPC)
        # global row index into the core-local flat [BPC*S, D] keys table
        idx_loc = idx[sl] + (np.arange(BPC, dtype=np.int32) * S)[:, None]
        # [BPC, CAP] -> [BPC, 128, CAP//128] partition-major for clean DMA
        idxT = np.ascontiguousarray(
            idx_loc.reshape(BPC, CAP // 128, 128).transpose(0, 2, 1)
        )
        in_maps.append({
            "keys": np.ascontiguousarray(keys[sl]),
            "queryT": np.ascontiguousarray(query[sl].T),
            "Wq": Wq,
            "Wk": Wk,
            "v": v,
            "idxT": idxT,
            "maskbias": np.ascontiguousarray(mb_sp[sl]),
        })

    res = bass_utils.run_bass_kernel_spmd(nc, in_maps, core_ids=list(range(NCORES)))

    u = np.concatenate([r["u"] for r in res.results], axis=0)        # [B, CAP]
    zp = np.concatenate([r["z"] for r in res.results], axis=0)       # [B, NG]
    cun = np.concatenate([r["cun"] for r in res.results], axis=0)    # [B, D]

    z = zp.sum(axis=1, dtype=np.float64).astype(np.float32)          # [B]
    a = np.zeros((B, S), dtype=np.float32)
    for j in range(B):
        a[j, idx[j, :counts[j]]] = u[j, :counts[j]] / z[j]
    c = cun / z[:, None]
    return (c, a)


# revision 14
# speedup vs baseline: 1.2951x; 1.2951x over previous
"""Bahdanau attention Trainium2 Bass kernel.

Problem: B=16, S=8192, Q_DIM=K_DIM=ATTN_DIM=512 (fp32).
    q = query @ Wq; k = keys @ Wk
    e = tanh(q + k) @ v;  e = where(mask==0, -inf, e)
    a = softmax(e);  c = a @ keys;  returns (c, a)

Sharding: data-parallel over batch. 8 cores x 2 batch rows each, weights
replicated. Each core computes unnormalized softmax numerators
u = exp(e_masked) (exact 0 at masked positions), partial sums z, and the
unnormalized context c_un = sum_s u_s keys_s. Final normalization (divide
by Z) happens on the host -- the cross-shard combine of the hinted design.

Matmuls run in fp32r (~tf32 precision, ~1.5 cycles/row warm). The BIR
verifier requires fp32r operands to come from rounding ops, so DMA'd keys
pass through a DVE round-copy; ACT writes tanh as fp32r directly.

HAM note: PE-transpose mode does not count as "busy" for the PE clock
gate, so a contiguous block of 16 transposes (~7us) re-throttles the PE
to 1.2 GHz every group. The group loop is software-pipelined: the
transposes of group g+1 are interleaved between the projection matmul
bursts of group g so every HAM window sees real matmul activity.

Device algorithm per core, per batch row, per group of 512 seq rows:
  - DMA 4 keys tiles [128, 512]; DVE round-copy -> fp32r
  - PE transpose -> keysT [4][128k, 512r]  (interleaved, see above)
  - proj psum[128a, 512r] = sum_kc Wk_chunk.T @ keysT_chunk
  - ACT tanh(psum + bias=qT[:, ac, b]) -> SBUF fp32r   (bias per-partition)
  - score psum_e[1,512] = sum_ac v_ac.T @ tanh_ac + ones.T @ maskbias(bf16)
  - ACT exp(psum_e) -> u tile, accum_out -> z partial; DMA u out
  - uT via 4 tiny fp32 transposes; DVE round-copy
  - c: psum_c[1,512] += uT_t.T @ keys_t   (accumulates over whole batch)
"""

import os

import numpy as np

B, S, D = 16, 8192, 512
NCORES = 8
BPC = B // NCORES        # batches per core
GROUP = 512              # seq rows per group
CAP = 4608               # capacity of gathered (unmasked) rows per batch
NG = CAP // GROUP        # 9 groups per batch
if os.environ.get("NG_OVERRIDE"):
    NG = int(os.environ["NG_OVERRIDE"])  # debug: process fewer groups
T = GROUP // 128         # 4 keys tiles per group
KC = D // 128            # 4 contraction chunks
AC = D // 128            # 4 attn-dim chunks
NEG = -1.0e30

_CACHE: dict = {}


def _build():
    if "nc" in _CACHE:
        return _CACHE["nc"]

    import concourse.bass as bass
    import concourse.mybir as mybir
    import concourse.tile as tile
    from concourse import bacc
    from concourse.masks import make_identity

    f32 = mybir.dt.float32
    f32r = mybir.dt.float32r
    bf16 = mybir.dt.bfloat16
    AF = mybir.ActivationFunctionType

    nc = bacc.Bacc("TRN2", target_bir_lowering=False, debug=False)

    i32 = mybir.dt.int32
    keys_d = nc.dram_tensor("keys", [BPC, S, D], f32r, kind="ExternalInput").ap()
    keys_flat = keys_d.rearrange("b s d -> (b s) d")
    qT_d = nc.dram_tensor("queryT", [D, BPC], f32, kind="ExternalInput").ap()
    wq_d = nc.dram_tensor("Wq", [D, D], f32, kind="ExternalInput").ap()
    wk_d = nc.dram_tensor("Wk", [D, D], f32r, kind="ExternalInput").ap()
    v_d = nc.dram_tensor("v", [D], f32r, kind="ExternalInput").ap()
    # idxT[b, p, j] = global row index (into keys_flat) of gathered row
    # j*128 + p of batch b; padding points at row b*S with maskbias -1e30.
    idx_d = nc.dram_tensor("idxT", [BPC, 128, CAP // 128], i32, kind="ExternalInput").ap()
    mb_d = nc.dram_tensor("maskbias", [BPC, CAP], bf16, kind="ExternalInput").ap()
    u_d = nc.dram_tensor("u", [BPC, CAP], f32, kind="ExternalOutput").ap()
    z_d = nc.dram_tensor("z", [BPC, NG], f32, kind="ExternalOutput").ap()
    c_d = nc.dram_tensor("cun", [BPC, D], f32, kind="ExternalOutput").ap()

    with tile.TileContext(nc) as tc:
        with (
            tc.tile_pool(name="singles", bufs=1) as singles,
            tc.tile_pool(name="keysp", bufs=10) as keysp,
            tc.tile_pool(name="keysrp", bufs=10) as keysrp,
            tc.tile_pool(name="ktp", bufs=10) as ktp,
            tc.tile_pool(name="thp", bufs=8) as thp,
            tc.tile_pool(name="mbp", bufs=4) as mbp,
            tc.tile_pool(name="utp", bufs=3) as utp,
            tc.tile_pool(name="up", bufs=4) as up,
            tc.tile_pool(name="zp", bufs=2) as zp,
            tc.tile_pool(name="coutp", bufs=2) as coutp,
            tc.tile_pool(name="idxp", bufs=2) as idxp,
            tc.tile_pool(name="ps_t", bufs=2, space="PSUM") as ps_t,
            tc.tile_pool(name="ps_p", bufs=2, space="PSUM") as ps_p,
            tc.tile_pool(name="ps_e", bufs=2, space="PSUM") as ps_e,
            tc.tile_pool(name="ps_u", bufs=1, space="PSUM") as ps_u,
            tc.tile_pool(name="ps_c", bufs=1, space="PSUM") as ps_c,
        ):
            # ---- preload constants ----
            ident = singles.tile([128, 128], f32)
            make_identity(nc, ident)
            ident_r = singles.tile([128, 128], f32r)
            nc.vector.tensor_copy(ident_r, ident)
            ones_bf = singles.tile([1, 1], bf16)
            nc.vector.memset(ones_bf, 1.0)

            wk_r = singles.tile([128, KC, D], f32r)
            nc.sync.dma_start(out=wk_r, in_=wk_d.rearrange("(kc p) a -> p kc a", p=128))
            wq_sb = singles.tile([128, KC, D], f32)
            nc.sync.dma_start(out=wq_sb, in_=wq_d.rearrange("(kc p) a -> p kc a", p=128))
            qTd_sb = singles.tile([128, KC, BPC], f32)
            nc.sync.dma_start(out=qTd_sb, in_=qT_d.rearrange("(kc p) b -> p kc b", p=128))
            v_r = singles.tile([128, AC], f32r)
            nc.sync.dma_start(out=v_r, in_=v_d.rearrange("(ac p) -> p ac", p=128))

            # ---- q projection (fp32, tiny): qT[a, b] = sum_k Wq[k,a] query[b,k] ----
            ps_q = ps_p.tile([128, AC * BPC], f32, tag="kp")
            for ac in range(AC):
                for kc in range(KC):
                    nc.tensor.matmul(
                        ps_q[:, ac * BPC:(ac + 1) * BPC],
                        lhsT=wq_sb[:, kc, ac * 128:(ac + 1) * 128],
                        rhs=qTd_sb[:, kc, :],
                        start=(kc == 0),
                        stop=(kc == KC - 1),
                    )
            qT_sb = singles.tile([128, AC * BPC], f32)
            nc.vector.tensor_copy(qT_sb, ps_q)

            # ---- pipelined main loop ----
            def emit_load(b, g, idx_sb):
                """Indirect-gather keys tiles + DVE round-cast for group g."""
                r0 = g * GROUP
                kr = []
                for t in range(T):
                    kw = keysp.tile([128, D], f32r, tag="keys", name=f"kw{b}_{g}_{t}")
                    nc.gpsimd.indirect_dma_start(
                        out=kw,
                        out_offset=None,
                        in_=keys_flat,
                        in_offset=bass.IndirectOffsetOnAxis(
                            ap=idx_sb[:, g * T + t:g * T + t + 1], axis=0
                        ),
                    )
                    krt = keysrp.tile([128, D], f32r, tag="keysr", name=f"kr{b}_{g}_{t}")
                    nc.vector.tensor_copy(krt, kw)
                    kr.append(krt)
                mb = mbp.tile([1, GROUP], bf16, tag="mb", name=f"mb{b}_{g}")
                nc.sync.dma_start(out=mb, in_=mb_d[b, None, r0:r0 + GROUP])
                return kr, mb

            def emit_transpose_block(kr, kc, b, g):
                """Transpose [128, 128] blocks of chunk kc -> kT SBUF tile."""
                pst = ps_t.tile([128, GROUP], f32r, tag="kT", name=f"pst{b}_{g}_{kc}")
                for t in range(T):
                    nc.tensor.transpose(
                        pst[:, t * 128:(t + 1) * 128],
                        kr[t][:, kc * 128:(kc + 1) * 128],
                        ident_r,
                    )
                kT = ktp.tile([128, GROUP], f32r, tag="kT_sb", name=f"kT{b}_{g}_{kc}")
                nc.vector.tensor_copy(kT, pst)
                return kT

            for b in range(BPC):
                z_sb = zp.tile([1, NG], f32, tag="z")
                psc = ps_c.tile([1, D], f32, tag="c")
                idx_sb = idxp.tile([128, CAP // 128], i32, tag="idx", name=f"idx{b}")
                nc.sync.dma_start(out=idx_sb, in_=idx_d[b])

                state = {}
                # prologue: load + transpose group 0
                state["load"] = emit_load(b, 0, idx_sb)
                state["kT"] = [
                    emit_transpose_block(state["load"][0], kc, b, 0) for kc in range(KC)
                ]

                for g in range(NG):
                    kr, mb = state["load"]
                    kTt = state["kT"]
                    if g + 1 < NG:
                        next_load = emit_load(b, g + 1, idx_sb)
                    else:
                        next_load = None
                    next_kT = []

                    # projection + tanh, with next group's transposes interleaved
                    th_tiles = []
                    for ac in range(AC):
                        if next_load is not None:
                            next_kT.append(
                                emit_transpose_block(next_load[0], ac, b, g + 1)
                            )
                        psp = ps_p.tile([128, GROUP], f32, tag="kp", name=f"kp{b}_{g}_{ac}")
                        for kc in range(KC):
                            nc.tensor.matmul(
                                psp,
                                lhsT=wk_r[:, kc, ac * 128:(ac + 1) * 128],
                                rhs=kTt[kc],
                                start=(kc == 0),
                                stop=(kc == KC - 1),
                            )
                        th = thp.tile([128, GROUP], f32r, tag="th", name=f"th{b}_{g}_{ac}")
                        nc.scalar.activation(
                            out=th, in_=psp, func=AF.Tanh,
                            bias=qT_sb[:, ac * BPC + b:ac * BPC + b + 1], scale=1.0,
                        )
                        th_tiles.append(th)

                    # scores
                    pse = ps_e.tile([1, GROUP], f32, tag="e", name=f"e{b}_{g}")
                    for ac in range(AC):
                        nc.tensor.matmul(
                            pse,
                            lhsT=v_r[:, ac:ac + 1],
                            rhs=th_tiles[ac],
                            start=(ac == 0),
                            stop=False,
                        )
                    nc.tensor.matmul(
                        pse, lhsT=ones_bf, rhs=mb,
                        start=False, stop=True, skip_group_check=True,
                    )

                    # u = exp(e) -> DMA out; z partial via accumulator
                    u_g = up.tile([1, GROUP], f32, tag="u", name=f"u{b}_{g}")
                    nc.scalar.activation(
                        out=u_g, in_=pse, func=AF.Exp,
                        accum_out=z_sb[:, g:g + 1],
                    )
                    nc.sync.dma_start(
                        out=u_d[b, None, g * GROUP:(g + 1) * GROUP], in_=u_g
                    )

                    # transpose u (tiny fp32 transposes) and accumulate context
                    psu = ps_u.tile([128, T], f32, tag="uT", name=f"uT{b}_{g}")
                    for t in range(T):
                        nc.tensor.transpose(
                            psu[:, t:t + 1],
                            u_g[:, t * 128:(t + 1) * 128],
                            ident[0:1, 0:1],
                        )
                    uT = utp.tile([128, T], f32r, tag="uT_sb", name=f"uTs{b}_{g}")
                    nc.vector.tensor_copy(uT, psu)
                    for t in range(T):
                        nc.tensor.matmul(
                            psc,
                            lhsT=uT[:, t:t + 1],
                            rhs=kr[t],
                            start=(g == 0 and t == 0),
                            stop=(g == NG - 1 and t == T - 1),
                        )

                    state["load"] = next_load
                    state["kT"] = next_kT

                # ---- batch epilogue ----
                c_sb = coutp.tile([1, D], f32, tag="cout")
                nc.vector.tensor_copy(c_sb, psc)
                nc.sync.dma_start(out=c_d[b, None, :], in_=c_sb)
                nc.sync.dma_start(out=z_d[b, None, :], in_=z_sb)

    nc.compile()
    _CACHE["nc"] = nc
    return nc


def kernel(query, keys, mask, Wq, Wk, v):
    import ml_dtypes
    from concourse import bass_utils

    query = np.asarray(query, dtype=np.float32)
    keys = np.asarray(keys, dtype=np.float32)
    mask = np.asarray(mask)
    Wq = np.ascontiguousarray(np.asarray(Wq, dtype=np.float32))
    Wk = np.ascontiguousarray(np.asarray(Wk, dtype=np.float32))
    v = np.ascontiguousarray(np.asarray(v, dtype=np.float32))

    # Gather plan: per batch row, the indices of unmasked positions, padded
    # to CAP with position 0 (given maskbias -1e30 so its u contribution is
    # exactly 0). Masked positions contribute exp(-inf)=0 in the reference
    # softmax, so skipping them is exact.
    idx_list, counts = [], []
    for j in range(B):
        nz = np.nonzero(mask[j])[0].astype(np.int32)
        n = len(nz)
        assert n <= CAP, f"unmasked count {n} exceeds kernel capacity {CAP}"
        counts.append(n)
        idx_list.append(np.pad(nz, (0, CAP - n)))
    idx = np.stack(idx_list)                                   # [B, CAP]
    mb_sp = np.zeros((B, CAP), dtype=ml_dtypes.bfloat16)
    for j in range(B):
        mb_sp[j, counts[j]:] = np.float32(NEG)

    nc = _build()

    in_maps = []
    for i in range(NCORES):
        sl = slice(i * BPC, (i + 1) * BPC)
        # global row index into the core-local flat [BPC*S, D] keys table
        idx_loc = idx[sl] + (np.arange(BPC, dtype=np.int32) * S)[:, None]
        # [BPC, CAP] -> [BPC, 128, CAP//128] partition-major for clean DMA
        idxT = np.ascontiguousarray(
            idx_loc.reshape(BPC, CAP // 128, 128).transpose(0, 2, 1)
        )
        in_maps.append({
            "keys": np.ascontiguousarray(keys[sl]),
            "queryT": np.ascontiguousarray(query[sl].T),
            "Wq": Wq,
            "Wk": Wk,
            "v": v,
            "idxT": idxT,
            "maskbias": np.ascontiguousarray(mb_sp[sl]),
        })

    res = bass_utils.run_bass_kernel_spmd(nc, in_maps, core_ids=list(range(NCORES)))

    u = np.concatenate([r["u"] for r in res.results], axis=0)        # [B, CAP]
    zp = np.concatenate([r["z"] for r in res.results], axis=0)       # [B, NG]
    cun = np.concatenate([r["cun"] for r in res.results], axis=0)    # [B, D]

    z = zp.sum(axis=1, dtype=np.float64).astype(np.float32)          # [B]
    a = np.zeros((B, S), dtype=np.float32)
    for j in range(B):
        a[j, idx[j, :counts[j]]] = u[j, :counts[j]] / z[j]
    c = cun / z[:, None]
    return (c, a)


# revision 15
# speedup vs baseline: 1.3018x; 1.0052x over previous
"""Bahdanau attention Trainium2 Bass kernel.

Problem: B=16, S=8192, Q_DIM=K_DIM=ATTN_DIM=512 (fp32).
    q = query @ Wq; k = keys @ Wk
    e = tanh(q + k) @ v;  e = where(mask==0, -inf, e)
    a = softmax(e);  c = a @ keys;  returns (c, a)

Sharding: data-parallel over batch. 8 cores x 2 batch rows each, weights
replicated. Each core computes unnormalized softmax numerators
u = exp(e_masked) (exact 0 at masked positions), partial sums z, and the
unnormalized context c_un = sum_s u_s keys_s. Final normalization (divide
by Z) happens on the host -- the cross-shard combine of the hinted design.

Matmuls run in fp32r (~tf32 precision, ~1.5 cycles/row warm). The BIR
verifier requires fp32r operands to come from rounding ops, so DMA'd keys
pass through a DVE round-copy; ACT writes tanh as fp32r directly.

HAM note: PE-transpose mode does not count as "busy" for the PE clock
gate, so a contiguous block of 16 transposes (~7us) re-throttles the PE
to 1.2 GHz every group. The group loop is software-pipelined: the
transposes of group g+1 are interleaved between the projection matmul
bursts of group g so every HAM window sees real matmul activity.

Device algorithm per core, per batch row, per group of 512 seq rows:
  - DMA 4 keys tiles [128, 512]; DVE round-copy -> fp32r
  - PE transpose -> keysT [4][128k, 512r]  (interleaved, see above)
  - proj psum[128a, 512r] = sum_kc Wk_chunk.T @ keysT_chunk
  - ACT tanh(psum + bias=qT[:, ac, b]) -> SBUF fp32r   (bias per-partition)
  - score psum_e[1,512] = sum_ac v_ac.T @ tanh_ac + ones.T @ maskbias(bf16)
  - ACT exp(psum_e) -> u tile, accum_out -> z partial; DMA u out
  - uT via 4 tiny fp32 transposes; DVE round-copy
  - c: psum_c[1,512] += uT_t.T @ keys_t   (accumulates over whole batch)
"""

import os

import numpy as np

B, S, D = 16, 8192, 512
NCORES = 8
BPC = B // NCORES        # batches per core
GROUP = 512              # seq rows per group
CAP = 4608               # capacity of gathered (unmasked) rows per batch
NG = CAP // GROUP        # 9 groups per batch
if os.environ.get("NG_OVERRIDE"):
    NG = int(os.environ["NG_OVERRIDE"])  # debug: process fewer groups
T = GROUP // 128         # 4 keys tiles per group
KC = D // 128            # 4 contraction chunks
AC = D // 128            # 4 attn-dim chunks
NEG = -1.0e30

_CACHE: dict = {}


def _build():
    if "nc" in _CACHE:
        return _CACHE["nc"]

    import concourse.bass as bass
    import concourse.mybir as mybir
    import concourse.tile as tile
    from concourse import bacc
    from concourse.masks import make_identity

    f32 = mybir.dt.float32
    f32r = mybir.dt.float32r
    bf16 = mybir.dt.bfloat16
    AF = mybir.ActivationFunctionType

    nc = bacc.Bacc("TRN2", target_bir_lowering=False, debug=False)

    i32 = mybir.dt.int32
    keys_d = nc.dram_tensor("keys", [BPC, S, D], f32r, kind="ExternalInput").ap()
    keys_flat = keys_d.rearrange("b s d -> (b s) d")
    qT_d = nc.dram_tensor("queryT", [D, BPC], f32, kind="ExternalInput").ap()
    wq_d = nc.dram_tensor("Wq", [D, D], f32, kind="ExternalInput").ap()
    wk_d = nc.dram_tensor("Wk", [D, D], f32r, kind="ExternalInput").ap()
    v_d = nc.dram_tensor("v", [D], f32r, kind="ExternalInput").ap()
    # idxT[b, p, j] = global row index (into keys_flat) of gathered row
    # j*128 + p of batch b; padding points at row b*S with maskbias -1e30.
    idx_d = nc.dram_tensor("idxT", [BPC, 128, CAP // 128], i32, kind="ExternalInput").ap()
    mb_d = nc.dram_tensor("maskbias", [BPC, CAP], bf16, kind="ExternalInput").ap()
    u_d = nc.dram_tensor("u", [BPC, CAP], f32, kind="ExternalOutput").ap()
    z_d = nc.dram_tensor("z", [BPC, NG], f32, kind="ExternalOutput").ap()
    c_d = nc.dram_tensor("cun", [BPC, D], f32, kind="ExternalOutput").ap()

    with tile.TileContext(nc) as tc:
        with (
            tc.tile_pool(name="singles", bufs=1) as singles,
            tc.tile_pool(name="keysp", bufs=10) as keysp,
            tc.tile_pool(name="keysrp", bufs=10) as keysrp,
            tc.tile_pool(name="ktp", bufs=10) as ktp,
            tc.tile_pool(name="thp", bufs=8) as thp,
            tc.tile_pool(name="mbp", bufs=4) as mbp,
            tc.tile_pool(name="utp", bufs=3) as utp,
            tc.tile_pool(name="up", bufs=4) as up,
            tc.tile_pool(name="zp", bufs=2) as zp,
            tc.tile_pool(name="coutp", bufs=2) as coutp,
            tc.tile_pool(name="idxp", bufs=2) as idxp,
            tc.tile_pool(name="ps_t", bufs=2, space="PSUM") as ps_t,
            tc.tile_pool(name="ps_p", bufs=2, space="PSUM") as ps_p,
            tc.tile_pool(name="ps_e", bufs=2, space="PSUM") as ps_e,
            tc.tile_pool(name="ps_u", bufs=1, space="PSUM") as ps_u,
            tc.tile_pool(name="ps_c", bufs=1, space="PSUM") as ps_c,
        ):
            # ---- preload constants ----
            # idx first: the first gathers are the longest dependency chain
            idx_tiles = []
            for b in range(BPC):
                idx_sb_b = idxp.tile([128, CAP // 128], i32, tag="idx", name=f"idx{b}")
                nc.sync.dma_start(out=idx_sb_b, in_=idx_d[b])
                idx_tiles.append(idx_sb_b)
            ident = singles.tile([128, 128], f32)
            make_identity(nc, ident)
            ident_r = singles.tile([128, 128], f32r)
            nc.vector.tensor_copy(ident_r, ident)
            ones_bf = singles.tile([1, 1], bf16)
            nc.vector.memset(ones_bf, 1.0)

            wk_r = singles.tile([128, KC, D], f32r)
            nc.sync.dma_start(out=wk_r, in_=wk_d.rearrange("(kc p) a -> p kc a", p=128))
            wq_sb = singles.tile([128, KC, D], f32)
            nc.sync.dma_start(out=wq_sb, in_=wq_d.rearrange("(kc p) a -> p kc a", p=128))
            qTd_sb = singles.tile([128, KC, BPC], f32)
            nc.sync.dma_start(out=qTd_sb, in_=qT_d.rearrange("(kc p) b -> p kc b", p=128))
            v_r = singles.tile([128, AC], f32r)
            nc.sync.dma_start(out=v_r, in_=v_d.rearrange("(ac p) -> p ac", p=128))

            # ---- q projection (fp32, tiny): qT[a, b] = sum_k Wq[k,a] query[b,k] ----
            ps_q = ps_p.tile([128, AC * BPC], f32, tag="kp")
            for ac in range(AC):
                for kc in range(KC):
                    nc.tensor.matmul(
                        ps_q[:, ac * BPC:(ac + 1) * BPC],
                        lhsT=wq_sb[:, kc, ac * 128:(ac + 1) * 128],
                        rhs=qTd_sb[:, kc, :],
                        start=(kc == 0),
                        stop=(kc == KC - 1),
                    )
            qT_sb = singles.tile([128, AC * BPC], f32)
            nc.vector.tensor_copy(qT_sb, ps_q)

            # PE warmup: harmless matmuls into a scratch bank keep the HAM
            # activity window busy until the first gathered tiles arrive.
            ps_warm = ps_e.tile([128, 128], f32, tag="e", name="warm")
            for _ in range(24):
                nc.tensor.matmul(ps_warm, lhsT=ident, rhs=ident, start=True, stop=True)

            # ---- pipelined main loop ----
            def emit_load(b, g, idx_sb):
                """Indirect-gather keys tiles + DVE round-cast for group g."""
                r0 = g * GROUP
                kr = []
                for t in range(T):
                    kw = keysp.tile([128, D], f32r, tag="keys", name=f"kw{b}_{g}_{t}")
                    nc.gpsimd.indirect_dma_start(
                        out=kw,
                        out_offset=None,
                        in_=keys_flat,
                        in_offset=bass.IndirectOffsetOnAxis(
                            ap=idx_sb[:, g * T + t:g * T + t + 1], axis=0
                        ),
                    )
                    krt = keysrp.tile([128, D], f32r, tag="keysr", name=f"kr{b}_{g}_{t}")
                    nc.vector.tensor_copy(krt, kw)
                    kr.append(krt)
                mb = mbp.tile([1, GROUP], bf16, tag="mb", name=f"mb{b}_{g}")
                nc.sync.dma_start(out=mb, in_=mb_d[b, None, r0:r0 + GROUP])
                return kr, mb

            def emit_transpose_block(kr, kc, b, g):
                """Transpose [128, 128] blocks of chunk kc -> kT SBUF tile."""
                pst = ps_t.tile([128, GROUP], f32r, tag="kT", name=f"pst{b}_{g}_{kc}")
                for t in range(T):
                    nc.tensor.transpose(
                        pst[:, t * 128:(t + 1) * 128],
                        kr[t][:, kc * 128:(kc + 1) * 128],
                        ident_r,
                    )
                kT = ktp.tile([128, GROUP], f32r, tag="kT_sb", name=f"kT{b}_{g}_{kc}")
                nc.vector.tensor_copy(kT, pst)
                return kT

            for b in range(BPC):
                z_sb = zp.tile([1, NG], f32, tag="z")
                psc = ps_c.tile([1, D], f32, tag="c")
                idx_sb = idx_tiles[b]

                state = {}
                # prologue: load + transpose group 0
                state["load"] = emit_load(b, 0, idx_sb)
                state["kT"] = [
                    emit_transpose_block(state["load"][0], kc, b, 0) for kc in range(KC)
                ]

                for g in range(NG):
                    kr, mb = state["load"]
                    kTt = state["kT"]
                    if g + 1 < NG:
                        next_load = emit_load(b, g + 1, idx_sb)
                    else:
                        next_load = None
                    next_kT = []

                    # projection + tanh, with next group's transposes interleaved
                    th_tiles = []
                    for ac in range(AC):
                        if next_load is not None:
                            next_kT.append(
                                emit_transpose_block(next_load[0], ac, b, g + 1)
                            )
                        psp = ps_p.tile([128, GROUP], f32, tag="kp", name=f"kp{b}_{g}_{ac}")
                        for kc in range(KC):
                            nc.tensor.matmul(
                                psp,
                                lhsT=wk_r[:, kc, ac * 128:(ac + 1) * 128],
                                rhs=kTt[kc],
                                start=(kc == 0),
                                stop=(kc == KC - 1),
                            )
                        th = thp.tile([128, GROUP], f32r, tag="th", name=f"th{b}_{g}_{ac}")
                        nc.scalar.activation(
                            out=th, in_=psp, func=AF.Tanh,
                            bias=qT_sb[:, ac * BPC + b:ac * BPC + b + 1], scale=1.0,
                        )
                        th_tiles.append(th)

                    # scores
                    pse = ps_e.tile([1, GROUP], f32, tag="e", name=f"e{b}_{g}")
                    for ac in range(AC):
                        nc.tensor.matmul(
                            pse,
                            lhsT=v_r[:, ac:ac + 1],
                            rhs=th_tiles[ac],
                            start=(ac == 0),
                            stop=False,
                        )
                    nc.tensor.matmul(
                        pse, lhsT=ones_bf, rhs=mb,
                        start=False, stop=True, skip_group_check=True,
                    )

                    # u = exp(e) -> DMA out; z partial via accumulator
                    u_g = up.tile([1, GROUP], f32, tag="u", name=f"u{b}_{g}")
                    nc.scalar.activation(
                        out=u_g, in_=pse, func=AF.Exp,
                        accum_out=z_sb[:, g:g + 1],
                    )
                    nc.sync.dma_start(
                        out=u_d[b, None, g * GROUP:(g + 1) * GROUP], in_=u_g
                    )

                    # transpose u (tiny fp32 transposes) and accumulate context
                    psu = ps_u.tile([128, T], f32, tag="uT", name=f"uT{b}_{g}")
                    for t in range(T):
                        nc.tensor.transpose(
                            psu[:, t:t + 1],
                            u_g[:, t * 128:(t + 1) * 128],
                            ident[0:1, 0:1],
                        )
                    uT = utp.tile([128, T], f32r, tag="uT_sb", name=f"uTs{b}_{g}")
                    nc.vector.tensor_copy(uT, psu)
                    for t in range(T):
                        nc.tensor.matmul(
                            psc,
                            lhsT=uT[:, t:t + 1],
                            rhs=kr[t],
                            start=(g == 0 and t == 0),
                            stop=(g == NG - 1 and t == T - 1),
                        )

                    state["load"] = next_load
                    state["kT"] = next_kT

                # ---- batch epilogue ----
                c_sb = coutp.tile([1, D], f32, tag="cout")
                nc.vector.tensor_copy(c_sb, psc)
                nc.sync.dma_start(out=c_d[b, None, :], in_=c_sb)
                nc.sync.dma_start(out=z_d[b, None, :], in_=z_sb)

    nc.compile()
    _CACHE["nc"] = nc
    return nc


def kernel(query, keys, mask, Wq, Wk, v):
    import ml_dtypes
    from concourse import bass_utils

    query = np.asarray(query, dtype=np.float32)
    keys = np.asarray(keys, dtype=np.float32)
    mask = np.asarray(mask)
    Wq = np.ascontiguousarray(np.asarray(Wq, dtype=np.float32))
    Wk = np.ascontiguousarray(np.asarray(Wk, dtype=np.float32))
    v = np.ascontiguousarray(np.asarray(v, dtype=np.float32))

    # Gather plan: per batch row, the indices of unmasked positions, padded
    # to CAP with position 0 (given maskbias -1e30 so its u contribution is
    # exactly 0). Masked positions contribute exp(-inf)=0 in the reference
    # softmax, so skipping them is exact.
    idx_list, counts = [], []
    for j in range(B):
        nz = np.nonzero(mask[j])[0].astype(np.int32)
        n = len(nz)
        assert n <= CAP, f"unmasked count {n} exceeds kernel capacity {CAP}"
        counts.append(n)
        idx_list.append(np.pad(nz, (0, CAP - n)))
    idx = np.stack(idx_list)                                   # [B, CAP]
    mb_sp = np.zeros((B, CAP), dtype=ml_dtypes.bfloat16)
    for j in range(B):
        mb_sp[j, counts[j]:] = np.float32(NEG)

    nc = _build()

    in_maps = []
    for i in range(NCORES):
        sl = slice(i * BPC, (i + 1) * BPC)
        # global row index into the core-local flat [BPC*S, D] keys table
        idx_loc = idx[sl] + (np.arange(BPC, dtype=np.int32) * S)[:, None]
        # [BPC, CAP] -> [BPC, 128, CAP//128] partition-major for clean DMA
        idxT = np.ascontiguousarray(
            idx_loc.reshape(BPC, CAP // 128, 128).transpose(0, 2, 1)
        )
        in_maps.append({
            "keys": np.ascontiguousarray(keys[sl]),
            "queryT": np.ascontiguousarray(query[sl].T),
            "Wq": Wq,
            "Wk": Wk,
            "v": v,
            "idxT": idxT,
            "maskbias": np.ascontiguousarray(mb_sp[sl]),
        })

    res = bass_utils.run_bass_kernel_spmd(nc, in_maps, core_ids=list(range(NCORES)))

    u = np.concatenate([r["u"] for r in res.results], axis=0)        # [B, CAP]
    zp = np.concatenate([r["z"] for r in res.results], axis=0)       # [B, NG]
    cun = np.concatenate([r["cun"] for r in res.results], axis=0)    # [B, D]

    z = zp.sum(axis=1, dtype=np.float64).astype(np.float32)          # [B]
    a = np.zeros((B, S), dtype=np.float32)
    for j in range(B):
        a[j, idx[j, :counts[j]]] = u[j, :counts[j]] / z[j]
    c = cun / z[:, None]
    return (c, a)


# revision 16
# speedup vs baseline: 1.4545x; 1.1173x over previous
"""Bahdanau attention Trainium2 Bass kernel.

Problem: B=16, S=8192, Q_DIM=K_DIM=ATTN_DIM=512 (fp32).
    q = query @ Wq; k = keys @ Wk
    e = tanh(q + k) @ v;  e = where(mask==0, -inf, e)
    a = softmax(e);  c = a @ keys;  returns (c, a)

Sharding: data-parallel over batch. 8 cores x 2 batch rows each, weights
replicated. Each core computes unnormalized softmax numerators
u = exp(e_masked) (exact 0 at masked positions), partial sums z, and the
unnormalized context c_un = sum_s u_s keys_s. Final normalization (divide
by Z) happens on the host -- the cross-shard combine of the hinted design.

Matmuls run in fp32r (~tf32 precision, ~1.5 cycles/row warm). The BIR
verifier requires fp32r operands to come from rounding ops, so DMA'd keys
pass through a DVE round-copy; ACT writes tanh as fp32r directly.

HAM note: PE-transpose mode does not count as "busy" for the PE clock
gate, so a contiguous block of 16 transposes (~7us) re-throttles the PE
to 1.2 GHz every group. The group loop is software-pipelined: the
transposes of group g+1 are interleaved between the projection matmul
bursts of group g so every HAM window sees real matmul activity.

Device algorithm per core, per batch row, per group of 512 seq rows:
  - DMA 4 keys tiles [128, 512]; DVE round-copy -> fp32r
  - PE transpose -> keysT [4][128k, 512r]  (interleaved, see above)
  - proj psum[128a, 512r] = sum_kc Wk_chunk.T @ keysT_chunk
  - ACT tanh(psum + bias=qT[:, ac, b]) -> SBUF fp32r   (bias per-partition)
  - score psum_e[1,512] = sum_ac v_ac.T @ tanh_ac + ones.T @ maskbias(bf16)
  - ACT exp(psum_e) -> u tile, accum_out -> z partial; DMA u out
  - uT via 4 tiny fp32 transposes; DVE round-copy
  - c: psum_c[1,512] += uT_t.T @ keys_t   (accumulates over whole batch)
"""

import os

import numpy as np

B, S, D = 16, 8192, 512
NCORES = 8
BPC = B // NCORES        # batches per core
GROUP = 512              # seq rows per group
CAP = 4608               # capacity of gathered (unmasked) rows per batch
NG = CAP // GROUP        # 9 groups per batch
if os.environ.get("NG_OVERRIDE"):
    NG = int(os.environ["NG_OVERRIDE"])  # debug: process fewer groups
T = GROUP // 128         # 4 keys tiles per group
KC = D // 128            # 4 contraction chunks
AC = D // 128            # 4 attn-dim chunks
NEG = -1.0e30

_CACHE: dict = {}


def _build():
    if "nc" in _CACHE:
        return _CACHE["nc"]

    import concourse.bass as bass
    import concourse.mybir as mybir
    import concourse.tile as tile
    from concourse import bacc
    from concourse.masks import make_identity

    f32 = mybir.dt.float32
    f32r = mybir.dt.float32r
    bf16 = mybir.dt.bfloat16
    AF = mybir.ActivationFunctionType

    nc = bacc.Bacc("TRN2", target_bir_lowering=False, debug=False)

    i32 = mybir.dt.int32
    keys_d = nc.dram_tensor("keys", [BPC, S, D], f32r, kind="ExternalInput").ap()
    keys_flat = keys_d.rearrange("b s d -> (b s) d")
    qT_d = nc.dram_tensor("queryT", [D, BPC], f32, kind="ExternalInput").ap()
    wq_d = nc.dram_tensor("Wq", [D, D], f32, kind="ExternalInput").ap()
    wk_d = nc.dram_tensor("Wk", [D, D], bf16, kind="ExternalInput").ap()
    v_d = nc.dram_tensor("v", [D], f32r, kind="ExternalInput").ap()
    # idxT[b, p, j] = global row index (into keys_flat) of gathered row
    # j*128 + p of batch b; padding points at row b*S with maskbias -1e30.
    idx_d = nc.dram_tensor("idxT", [BPC, 128, CAP // 128], i32, kind="ExternalInput").ap()
    mb_d = nc.dram_tensor("maskbias", [BPC, CAP], bf16, kind="ExternalInput").ap()
    u_d = nc.dram_tensor("u", [BPC, CAP], f32, kind="ExternalOutput").ap()
    z_d = nc.dram_tensor("z", [BPC, NG], f32, kind="ExternalOutput").ap()
    c_d = nc.dram_tensor("cun", [BPC, D], f32, kind="ExternalOutput").ap()

    with tile.TileContext(nc) as tc:
        with (
            tc.tile_pool(name="singles", bufs=1) as singles,
            tc.tile_pool(name="keysp", bufs=10) as keysp,
            tc.tile_pool(name="keysrp", bufs=10) as keysrp,
            tc.tile_pool(name="ktp", bufs=10) as ktp,
            tc.tile_pool(name="thp", bufs=8) as thp,
            tc.tile_pool(name="mbp", bufs=4) as mbp,
            tc.tile_pool(name="utp", bufs=3) as utp,
            tc.tile_pool(name="up", bufs=4) as up,
            tc.tile_pool(name="zp", bufs=2) as zp,
            tc.tile_pool(name="coutp", bufs=2) as coutp,
            tc.tile_pool(name="idxp", bufs=2) as idxp,
            tc.tile_pool(name="ps_t", bufs=2, space="PSUM") as ps_t,
            tc.tile_pool(name="ps_p", bufs=2, space="PSUM") as ps_p,
            tc.tile_pool(name="ps_e", bufs=2, space="PSUM") as ps_e,
            tc.tile_pool(name="ps_u", bufs=1, space="PSUM") as ps_u,
            tc.tile_pool(name="ps_c", bufs=1, space="PSUM") as ps_c,
        ):
            # ---- preload constants ----
            # idx first: the first gathers are the longest dependency chain
            idx_tiles = []
            for b in range(BPC):
                idx_sb_b = idxp.tile([128, CAP // 128], i32, tag="idx", name=f"idx{b}")
                nc.sync.dma_start(out=idx_sb_b, in_=idx_d[b])
                idx_tiles.append(idx_sb_b)
            ident = singles.tile([128, 128], f32)
            make_identity(nc, ident)
            ident_r = singles.tile([128, 128], bf16)
            nc.vector.tensor_copy(ident_r, ident)
            ones_bf = singles.tile([1, 1], bf16)
            nc.vector.memset(ones_bf, 1.0)

            wk_r = singles.tile([128, KC, D], bf16)
            nc.sync.dma_start(out=wk_r, in_=wk_d.rearrange("(kc p) a -> p kc a", p=128))
            wq_sb = singles.tile([128, KC, D], f32)
            nc.sync.dma_start(out=wq_sb, in_=wq_d.rearrange("(kc p) a -> p kc a", p=128))
            qTd_sb = singles.tile([128, KC, BPC], f32)
            nc.sync.dma_start(out=qTd_sb, in_=qT_d.rearrange("(kc p) b -> p kc b", p=128))
            v_r = singles.tile([128, AC], f32r)
            nc.sync.dma_start(out=v_r, in_=v_d.rearrange("(ac p) -> p ac", p=128))

            # ---- q projection (fp32, tiny): qT[a, b] = sum_k Wq[k,a] query[b,k] ----
            ps_q = ps_p.tile([128, AC * BPC], f32, tag="kp")
            for ac in range(AC):
                for kc in range(KC):
                    nc.tensor.matmul(
                        ps_q[:, ac * BPC:(ac + 1) * BPC],
                        lhsT=wq_sb[:, kc, ac * 128:(ac + 1) * 128],
                        rhs=qTd_sb[:, kc, :],
                        start=(kc == 0),
                        stop=(kc == KC - 1),
                    )
            qT_sb = singles.tile([128, AC * BPC], f32)
            nc.vector.tensor_copy(qT_sb, ps_q)

            # PE warmup: harmless matmuls into a scratch bank keep the HAM
            # activity window busy until the first gathered tiles arrive.
            ps_warm = ps_e.tile([128, 128], f32, tag="e", name="warm")
            for _ in range(24):
                nc.tensor.matmul(ps_warm, lhsT=ident, rhs=ident, start=True, stop=True)

            # ---- pipelined main loop ----
            def emit_load(b, g, idx_sb):
                """Indirect-gather keys tiles + DVE round-cast for group g."""
                r0 = g * GROUP
                kr = []
                for t in range(T):
                    kw = keysp.tile([128, D], f32r, tag="keys", name=f"kw{b}_{g}_{t}")
                    nc.gpsimd.indirect_dma_start(
                        out=kw,
                        out_offset=None,
                        in_=keys_flat,
                        in_offset=bass.IndirectOffsetOnAxis(
                            ap=idx_sb[:, g * T + t:g * T + t + 1], axis=0
                        ),
                    )
                    krt = keysrp.tile([128, D], bf16, tag="keysr", name=f"kr{b}_{g}_{t}")
                    nc.vector.tensor_copy(krt, kw)
                    kr.append(krt)
                mb = mbp.tile([1, GROUP], bf16, tag="mb", name=f"mb{b}_{g}")
                nc.sync.dma_start(out=mb, in_=mb_d[b, None, r0:r0 + GROUP])
                return kr, mb

            def emit_transpose_block(kr, kc, b, g):
                """Transpose [128, 128] blocks of chunk kc -> kT SBUF tile."""
                pst = ps_t.tile([128, GROUP], bf16, tag="kT", name=f"pst{b}_{g}_{kc}")
                for t in range(T):
                    nc.tensor.transpose(
                        pst[:, t * 128:(t + 1) * 128],
                        kr[t][:, kc * 128:(kc + 1) * 128],
                        ident_r,
                    )
                kT = ktp.tile([128, GROUP], bf16, tag="kT_sb", name=f"kT{b}_{g}_{kc}")
                nc.vector.tensor_copy(kT, pst)
                return kT

            for b in range(BPC):
                z_sb = zp.tile([1, NG], f32, tag="z")
                psc = ps_c.tile([1, D], f32, tag="c")
                idx_sb = idx_tiles[b]

                state = {}
                # prologue: load + transpose group 0
                state["load"] = emit_load(b, 0, idx_sb)
                state["kT"] = [
                    emit_transpose_block(state["load"][0], kc, b, 0) for kc in range(KC)
                ]

                for g in range(NG):
                    kr, mb = state["load"]
                    kTt = state["kT"]
                    if g + 1 < NG:
                        next_load = emit_load(b, g + 1, idx_sb)
                    else:
                        next_load = None
                    next_kT = []

                    # projection + tanh, with next group's transposes interleaved
                    th_tiles = []
                    for ac in range(AC):
                        if next_load is not None:
                            next_kT.append(
                                emit_transpose_block(next_load[0], ac, b, g + 1)
                            )
                        psp = ps_p.tile([128, GROUP], f32, tag="kp", name=f"kp{b}_{g}_{ac}")
                        for kc in range(KC):
                            nc.tensor.matmul(
                                psp,
                                lhsT=wk_r[:, kc, ac * 128:(ac + 1) * 128],
                                rhs=kTt[kc],
                                start=(kc == 0),
                                stop=(kc == KC - 1),
                            )
                        th = thp.tile([128, GROUP], f32r, tag="th", name=f"th{b}_{g}_{ac}")
                        nc.scalar.activation(
                            out=th, in_=psp, func=AF.Tanh,
                            bias=qT_sb[:, ac * BPC + b:ac * BPC + b + 1], scale=1.0,
                        )
                        th_tiles.append(th)

                    # scores
                    pse = ps_e.tile([1, GROUP], f32, tag="e", name=f"e{b}_{g}")
                    for ac in range(AC):
                        nc.tensor.matmul(
                            pse,
                            lhsT=v_r[:, ac:ac + 1],
                            rhs=th_tiles[ac],
                            start=(ac == 0),
                            stop=False,
                        )
                    nc.tensor.matmul(
                        pse, lhsT=ones_bf, rhs=mb,
                        start=False, stop=True, skip_group_check=True,
                    )

                    # u = exp(e) -> DMA out; z partial via accumulator
                    u_g = up.tile([1, GROUP], f32, tag="u", name=f"u{b}_{g}")
                    nc.scalar.activation(
                        out=u_g, in_=pse, func=AF.Exp,
                        accum_out=z_sb[:, g:g + 1],
                    )
                    nc.sync.dma_start(
                        out=u_d[b, None, g * GROUP:(g + 1) * GROUP], in_=u_g
                    )

                    # transpose u (tiny fp32 transposes) and accumulate context
                    psu = ps_u.tile([128, T], f32, tag="uT", name=f"uT{b}_{g}")
                    for t in range(T):
                        nc.tensor.transpose(
                            psu[:, t:t + 1],
                            u_g[:, t * 128:(t + 1) * 128],
                            ident[0:1, 0:1],
                        )
                    uT = utp.tile([128, T], bf16, tag="uT_sb", name=f"uTs{b}_{g}")
                    nc.vector.tensor_copy(uT, psu)
                    for t in range(T):
                        nc.tensor.matmul(
                            psc,
                            lhsT=uT[:, t:t + 1],
                            rhs=kr[t],
                            start=(g == 0 and t == 0),
                            stop=(g == NG - 1 and t == T - 1),
                        )

                    state["load"] = next_load
                    state["kT"] = next_kT

                # ---- batch epilogue ----
                c_sb = coutp.tile([1, D], f32, tag="cout")
                nc.vector.tensor_copy(c_sb, psc)
                nc.sync.dma_start(out=c_d[b, None, :], in_=c_sb)
                nc.sync.dma_start(out=z_d[b, None, :], in_=z_sb)

    nc.compile()
    _CACHE["nc"] = nc
    return nc


def kernel(query, keys, mask, Wq, Wk, v):
    import ml_dtypes
    from concourse import bass_utils

    query = np.asarray(query, dtype=np.float32)
    keys = np.asarray(keys, dtype=np.float32)
    mask = np.asarray(mask)
    Wq = np.ascontiguousarray(np.asarray(Wq, dtype=np.float32))
    Wk = np.ascontiguousarray(np.asarray(Wk, dtype=np.float32).astype(ml_dtypes.bfloat16))
    v = np.ascontiguousarray(np.asarray(v, dtype=np.float32))

    # Gather plan: per batch row, the indices of unmasked positions, padded
    # to CAP with position 0 (given maskbias -1e30 so its u contribution is
    # exactly 0). Masked positions contribute exp(-inf)=0 in the reference
    # softmax, so skipping them is exact.
    idx_list, counts = [], []
    for j in range(B):
        nz = np.nonzero(mask[j])[0].astype(np.int32)
        n = len(nz)
        assert n <= CAP, f"unmasked count {n} exceeds kernel capacity {CAP}"
        counts.append(n)
        idx_list.append(np.pad(nz, (0, CAP - n)))
    idx = np.stack(idx_list)                                   # [B, CAP]
    mb_sp = np.zeros((B, CAP), dtype=ml_dtypes.bfloat16)
    for j in range(B):
        mb_sp[j, counts[j]:] = np.float32(NEG)

    nc = _build()

    in_maps = []
    for i in range(NCORES):
        sl = slice(i * BPC, (i + 1) * BPC)
        # global row index into the core-local flat [BPC*S, D] keys table
        idx_loc = idx[sl] + (np.arange(BPC, dtype=np.int32) * S)[:, None]
        # [BPC, CAP] -> [BPC, 128, CAP//128] partition-major for clean DMA
        idxT = np.ascontiguousarray(
            idx_loc.reshape(BPC, CAP // 128, 128).transpose(0, 2, 1)
        )
        in_maps.append({
            "keys": np.ascontiguousarray(keys[sl]),
            "queryT": np.ascontiguousarray(query[sl].T),
            "Wq": Wq,
            "Wk": Wk,
            "v": v,
            "idxT": idxT,
            "maskbias": np.ascontiguousarray(mb_sp[sl]),
        })

    res = bass_utils.run_bass_kernel_spmd(nc, in_maps, core_ids=list(range(NCORES)))

    u = np.concatenate([r["u"] for r in res.results], axis=0)        # [B, CAP]
    zp = np.concatenate([r["z"] for r in res.results], axis=0)       # [B, NG]
    cun = np.concatenate([r["cun"] for r in res.results], axis=0)    # [B, D]

    z = zp.sum(axis=1, dtype=np.float64).astype(np.float32)          # [B]
    a = np.zeros((B, S), dtype=np.float32)
    for j in range(B):
        a[j, idx[j, :counts[j]]] = u[j, :counts[j]] / z[j]
    c = cun / z[:, None]
    return (c, a)
